# revision 20
# baseline (speedup 1.0000x reference)
"""Single-head attention (B=4, S=2048, D=1024) on 8 trn2 NeuronCores.

Sharding: core = batch*2 + kv_half.  Each core receives ONLY its kv-half
of x^T (xh = x[b].T[:, half]) and computes
  Qown = xh @ Wq^T + bq        (its 1024 own-half queries)
  K    = xh @ Wk^T + bk        (its 1024 keys)
  V    = xh @ Wv^T             (bv folded in on host)
The pair cores exchange Q^T halves with a pairwise HBM AllGather (2MB)
that overlaps the K/V projections; each core then attends all 2048
queries against its kv-half:
  ST = K @ Q^T, PT = exp(ST/32), l = ones^T @ PT, accT = V'slices @ PT
Queries are processed in LOCAL order ([own half | peer half]) so the ST
pass starts without waiting on the collective; the host swaps the odd
cores' output halves back and merges:
  out[b] = (acc0 + acc1) / (l0 + l1) + bv.

All tensors are bf16 on device (fp32 PSUM); bf16 matmuls run at full PE
rate and everything stays SBUF-resident.  DMAs are issued on the sync
ring only (ACT-ring DMAs wedge the device) and are emitted in deadline
order — the SP engine processes them serially at ~185GB/s, so program
order is the schedule.  The peer-half gather-in DMAs read
qall_d[1 - parity] through a runtime register index (bass.ts) so one
NEFF serves both pair members.
"""

import sys
import numpy as np

for _p in ("/root/.axon_site/_ro/trn_rl_repo", "/opt/trn_rl_repo"):
    if _p not in sys.path:
        sys.path.append(_p)

import ml_dtypes
import concourse.bass as bass
import concourse.tile as tile
from concourse import bacc, mybir
from concourse.bass_utils import run_bass_kernel_spmd

F32 = mybir.dt.float32
BF16 = mybir.dt.bfloat16
BF = ml_dtypes.bfloat16

B, S, D = 4, 2048, 1024
H = S // 2          # kv-half size (1024)
DT = D // 128       # 8 contraction tiles
ET = D // 128       # 8 output-dim tiles
XCH = H // 512      # 2 column chunks of xh
NCH = S // 512      # 4 query chunks
N_CORES = 8

_compiled = None


def _build():
    nc = bacc.Bacc("TRN2", target_bir_lowering=False, debug=False,
                   num_devices=N_CORES)

    xh = nc.dram_tensor("xh", [D, H], BF16, kind="ExternalInput").ap()
    wqt = nc.dram_tensor("wqt", [D, D], BF16, kind="ExternalInput").ap()
    wkt = nc.dram_tensor("wkt", [D, D], BF16, kind="ExternalInput").ap()
    wvt = nc.dram_tensor("wvt", [D, D], BF16, kind="ExternalInput").ap()
    bq1 = nc.dram_tensor("bq1", [D], F32, kind="ExternalInput").ap()
    bk1 = nc.dram_tensor("bk1", [D], F32, kind="ExternalInput").ap()
    ones = nc.dram_tensor("ones", [128, 1], BF16, kind="ExternalInput").ap()

    accT_d = nc.dram_tensor("accT_d", [D, S], BF16, kind="ExternalOutput").ap()
    l_d = nc.dram_tensor("l_d", [S], F32, kind="ExternalOutput").ap()

    Ident = mybir.ActivationFunctionType.Identity
    Exp = mybir.ActivationFunctionType.Exp

    with tile.TileContext(nc) as tc:
        with (
            tc.tile_pool(name="const", bufs=1) as const,
            tc.tile_pool(name="wpool", bufs=1) as wpool,
            tc.tile_pool(name="xpool", bufs=1) as xpool,
            tc.tile_pool(name="qtp", bufs=1) as qtp,
            tc.tile_pool(name="ktp", bufs=1) as ktp,
            tc.tile_pool(name="vvp", bufs=1) as vvp,
            tc.tile_pool(name="ptp", bufs=1) as ptp,
            tc.tile_pool(name="stg", bufs=4) as stg,
            tc.tile_pool(name="lst", bufs=2) as lst,
        ):
            ones_sb = const.tile([128, 1], BF16, name="ones_sb")
            nc.sync.dma_start(out=ones_sb, in_=ones)
            bq_sb = const.tile([128, ET], F32, name="bq_sb")
            nc.sync.dma_start(
                out=bq_sb,
                in_=bass.AP(tensor=bq1.tensor, offset=0,
                            ap=[[1, 128], [128, ET]]))
            bk_sb = const.tile([128, ET], F32, name="bk_sb")
            nc.sync.dma_start(
                out=bk_sb,
                in_=bass.AP(tensor=bk1.tensor, offset=0,
                            ap=[[1, 128], [128, ET]]))

            wq_t = [wpool.tile([128, D], BF16, name=f"wq{dt}")
                    for dt in range(DT)]
            wk_t = [wpool.tile([128, D], BF16, name=f"wk{dt}")
                    for dt in range(DT)]
            wv_t = [wpool.tile([128, D], BF16, name=f"wv{dt}")
                    for dt in range(DT)]
            xct = [[xpool.tile([128, 512], BF16, name=f"x{c}_{dt}")
                    for dt in range(DT)] for c in range(XCH)]
            qt_c = [qtp.tile([128, ET, 512], BF16, name=f"qt{c}")
                    for c in range(NCH)]
            kt_c = [ktp.tile([128, ET, 512], BF16, name=f"kt{cc}")
                    for cc in range(XCH)]
            v_c = [vvp.tile([128, 4, D], BF16, name=f"v{cc}")
                   for cc in range(XCH)]
            pt_b = [ptp.tile([128, 8, 512], BF16, name=f"pt{blk}")
                    for blk in range(NCH)]

            # DRAM bounce buffers for the pairwise Q AllGather
            with tc.tile_pool(name="dram", bufs=1, space="DRAM") as dram:
                qown_d = dram.tile([D, H], BF16, name="qown_d")
                qall_d = dram.tile([2, D, H], BF16, name="qall_d")

            # core parity (runtime register) for the predicated gather-in
            parity = nc.sync.partition_id() & 1
            not_parity = 1 - parity

            # ---- input DMAs in deadline order (SP processes serially,
            # ~185GB/s; program order IS the schedule) ----
            def dma_w(wt, src, dt):
                for p in range(2):
                    nc.sync.dma_start(
                        out=wt[dt][:, p * 512:(p + 1) * 512],
                        in_=src[dt * 128:(dt + 1) * 128,
                                p * 512:(p + 1) * 512])

            for dt in range(DT):
                dma_w(wq_t, wqt, dt)
                nc.sync.dma_start(out=xct[0][dt],
                                  in_=xh[dt * 128:(dt + 1) * 128, 0:512])
            for dt in range(DT):
                nc.sync.dma_start(out=xct[1][dt],
                                  in_=xh[dt * 128:(dt + 1) * 128, 512:1024])
            for dt in range(DT):
                dma_w(wk_t, wkt, dt)

            # (no PE warmup: the junk matmuls themselves run at the cold
            # 1.2GHz clock and outlast the initial DMA wait, so letting
            # phase A absorb the ~3.4us HAM ramp is a net ~3us win)

            # ================= Phases A/B: projections =================
            with tc.tile_pool(name="psAB", bufs=8, space="PSUM") as psA:
                # ---- Phase A: Q projection for the OWN kv-half queries;
                # each chunk is bounced to DRAM for the pair AllGather.
                for c in range(XCH):
                    ps = [psA.tile([128, 512], F32, tag="ps",
                                   name=f"psq{c}_{e}") for e in range(ET)]
                    for dt in range(DT):
                        for e in range(ET):
                            nc.tensor.matmul(
                                ps[e], wq_t[dt][:, e * 128:(e + 1) * 128],
                                xct[c][dt],
                                start=(dt == 0), stop=(dt == DT - 1))
                            # inline drain right after each e's stop-matmul
                            # so the banks free up before the next chunk
                            if dt == DT - 1:
                                nc.scalar.activation(
                                    qt_c[c][:, e, :], ps[e], Ident,
                                    bias=bq_sb[:, e:e + 1], scale=1.0)
                    for e in range(ET):
                        nc.sync.dma_start(
                            out=qown_d[e * 128:(e + 1) * 128,
                                       c * 512:(c + 1) * 512],
                            in_=qt_c[c][:, e, :])

                # wv loads slot between the qown chunks' act-waits
                for dt in range(DT):
                    dma_w(wv_t, wvt, dt)

                # Pairwise AllGather: qall_d = [even core's Q^T half,
                # odd core's Q^T half]; overlaps the K/V projections.
                nc.gpsimd.collective_compute(
                    "AllGather", mybir.AluOpType.bypass,
                    replica_groups=[[0, 1], [2, 3], [4, 5], [6, 7]],
                    ins=[qown_d.opt()], outs=[qall_d.opt()])

                # peer half -> qt chunks 2,3.  The source half index is a
                # runtime register (1-parity): even cores read qall_d[1],
                # odd cores qall_d[0].  A single DMA per slice (vs a
                # cond-predicated pair) avoids WAW serialization.
                for c2 in range(2):
                    for e in range(ET):
                        src = qall_d[bass.ts(not_parity, 1),
                                     e * 128:(e + 1) * 128,
                                     c2 * 512:(c2 + 1) * 512]
                        nc.sync.dma_start(out=qt_c[2 + c2][:, e, :],
                                          in_=src)

                # ---- Phase B: K ----
                for cc in range(XCH):
                    ps = [psA.tile([128, 512], F32, tag="ps",
                                   name=f"psk{cc}_{e}") for e in range(ET)]
                    for dt in range(DT):
                        for e in range(ET):
                            nc.tensor.matmul(
                                ps[e], wk_t[dt][:, e * 128:(e + 1) * 128],
                                xct[cc][dt],
                                start=(dt == 0), stop=(dt == DT - 1))
                            if dt == DT - 1:
                                nc.scalar.activation(
                                    kt_c[cc][:, e, :], ps[e], Ident,
                                    bias=bk_sb[:, e:e + 1], scale=1.0)

                # ---- Phase B: V (x-slices stationary, wv moving) ----
                for cc in range(XCH):
                    for j2 in range(4):
                        pv = [psA.tile([128, 512], F32, tag="ps",
                                       name=f"psv{cc}_{j2}_{ec}")
                              for ec in range(2)]
                        for dt in range(DT):
                            for ec in range(2):
                                nc.tensor.matmul(
                                    pv[ec],
                                    xct[cc][dt][:, j2 * 128:(j2 + 1) * 128],
                                    wv_t[dt][:, ec * 512:(ec + 1) * 512],
                                    start=(dt == 0), stop=(dt == DT - 1))
                        for ec in range(2):
                            nc.vector.tensor_copy(
                                v_c[cc][:, j2, ec * 512:(ec + 1) * 512],
                                pv[ec])

            # ================= Phase C: attention =================
            with tc.tile_pool(name="psC", bufs=6, space="PSUM") as psC:
                # l(blk) = ones^T @ PT(blk).  Emitted one block late so its
                # exp dependencies are long done (no PE stall), and it
                # shares the "ps" tag so pool-slot rotation forces it to
                # execute inside the ST stream instead of drifting to the
                # kernel tail.
                def emit_l(blk):
                    lp = psC.tile([128, 512], F32, tag="ps",
                                  name=f"lp{blk}")
                    for j in range(8):
                        nc.tensor.matmul(
                            lp[0:1, :], ones_sb, pt_b[blk][:, j, :],
                            start=(j == 0), stop=(j == 7))
                    l_st = lst.tile([1, 512], F32, tag="l",
                                    name=f"lst{blk}")
                    nc.vector.tensor_copy(l_st, lp[0:1, :])
                    nc.sync.dma_start(
                        out=l_d[blk * 512:(blk + 1) * 512], in_=l_st)

                # ---- ST = K @ Q^T, PT = exp(ST/32); queries in LOCAL
                # order: blocks 0,1 = own half (no collective dep),
                # blocks 2,3 = peer half.
                for blk in range(NCH):
                    for j in range(8):
                        cc, jj = divmod(j, 4)
                        sp = psC.tile([128, 512], F32, tag="ps",
                                      name=f"sp{blk}_{j}")
                        for e in range(ET):
                            nc.tensor.matmul(
                                sp, kt_c[cc][:, e, jj * 128:(jj + 1) * 128],
                                qt_c[blk][:, e, :],
                                start=(e == 0), stop=(e == ET - 1))
                        nc.scalar.activation(
                            pt_b[blk][:, j, :], sp, Exp,
                            bias=0.0, scale=float(1.0 / 32.0))
                    if blk >= 1:
                        emit_l(blk - 1)
                emit_l(NCH - 1)

                # ---- accT = V^T-slices @ PT ----
                # blk-outer so each PSUM bank drains right after its
                # 8-chain (spreads the DVE copies and shortens the tail)
                for e in range(ET):
                    for blk in range(NCH):
                        av = psC.tile([128, 512], F32, tag="ps",
                                      name=f"av{e}_{blk}")
                        for j in range(8):
                            cc, jj = divmod(j, 4)
                            nc.tensor.matmul(
                                av,
                                v_c[cc][:, jj, e * 128:(e + 1) * 128],
                                pt_b[blk][:, j, :],
                                start=(j == 0), stop=(j == 7))
                        st_t = stg.tile([128, 512], BF16, tag="stg",
                                        name=f"acc{e}_{blk}")
                        nc.vector.tensor_copy(st_t, av)
                        nc.sync.dma_start(
                            out=accT_d[e * 128:(e + 1) * 128,
                                       blk * 512:(blk + 1) * 512],
                            in_=st_t)

    nc.compile()
    return nc


def _get_compiled():
    global _compiled
    if _compiled is None:
        _compiled = _build()
    return _compiled


def run_sharded(inputs, **run_kwargs):
    """Build per-core in_maps, run SPMD, return BassKernelResults."""
    x = np.ascontiguousarray(inputs["x"], dtype=np.float32)
    Wq = np.asarray(inputs["Wq"], dtype=np.float32)
    Wk = np.asarray(inputs["Wk"], dtype=np.float32)
    Wv = np.asarray(inputs["Wv"], dtype=np.float32)
    bq = np.asarray(inputs["bq"], dtype=np.float32)
    bk = np.asarray(inputs["bk"], dtype=np.float32)

    nc = _get_compiled()

    wqt = np.ascontiguousarray(Wq.T).astype(BF)
    wkt = np.ascontiguousarray(Wk.T).astype(BF)
    wvt = np.ascontiguousarray(Wv.T).astype(BF)
    ones = np.ones((128, 1), dtype=np.float32).astype(BF)

    in_maps = []
    for core in range(N_CORES):
        b, h = divmod(core, 2)
        xhb = x[b].T[:, h * H:(h + 1) * H]            # [D, H] own kv-half
        in_maps.append(dict(xh=np.ascontiguousarray(xhb).astype(BF),
                            wqt=wqt, wkt=wkt, wvt=wvt,
                            bq1=bq, bk1=bk, ones=ones))

    return run_bass_kernel_spmd(nc, in_maps, core_ids=list(range(N_CORES)),
                                **run_kwargs)


def kernel(**inputs):
    bv = np.asarray(inputs["bv"], dtype=np.float64)
    res = run_sharded(inputs)

    out = np.empty((B, S, D), dtype=np.float32)
    for b in range(B):
        r0 = res.results[b * 2]
        r1 = res.results[b * 2 + 1]
        a0 = np.asarray(r0["accT_d"], dtype=np.float64)       # [D, S]
        a1 = np.asarray(r1["accT_d"], dtype=np.float64)
        # each core's output query order is [own half | peer half];
        # odd cores' halves are swapped relative to natural order
        a1 = np.concatenate([a1[:, H:], a1[:, :H]], axis=1)
        l0 = np.asarray(r0["l_d"], dtype=np.float64)
        l1 = np.asarray(r1["l_d"], dtype=np.float64)
        l1 = np.concatenate([l1[H:], l1[:H]])
        num = a0.T + a1.T
        den = (l0 + l1)[:, None]
        out[b] = (num / den + bv[None, :]).astype(np.float32)
    return out


# revision 24
# speedup vs baseline: 1.0005x; 1.0005x over previous
"""Single-head attention (B=4, S=2048, D=1024) on 8 trn2 NeuronCores.

Sharding: core = batch*2 + kv_half.  Each core receives ONLY its kv-half
of x^T (xh = x[b].T[:, half]) and computes
  Qown = xh @ Wq^T + bq        (its 1024 own-half queries)
  K    = xh @ Wk^T + bk        (its 1024 keys)
  V    = xh @ Wv^T             (bv folded in on host)
The pair cores exchange Q^T halves with a pairwise HBM AllGather (2MB)
that overlaps the K/V projections; each core then attends all 2048
queries against its kv-half:
  ST = K @ Q^T, PT = exp(ST/32), l = ones^T @ PT, accT = V'slices @ PT
Queries are processed in LOCAL order ([own half | peer half]) so the ST
pass starts without waiting on the collective; the host swaps the odd
cores' output halves back and merges:
  out[b] = (acc0 + acc1) / (l0 + l1) + bv.

All tensors are bf16 on device (fp32 PSUM); bf16 matmuls run at full PE
rate and everything stays SBUF-resident.  DMAs are issued on the sync
ring only (ACT-ring DMAs wedge the device) and are emitted in deadline
order — the SP engine processes them serially at ~185GB/s, so program
order is the schedule.  The peer-half gather-in DMAs read
qall_d[1 - parity] through a runtime register index (bass.ts) so one
NEFF serves both pair members.
"""

import sys
import numpy as np

for _p in ("/root/.axon_site/_ro/trn_rl_repo", "/opt/trn_rl_repo"):
    if _p not in sys.path:
        sys.path.append(_p)

import ml_dtypes
import concourse.bass as bass
import concourse.tile as tile
from concourse import bacc, mybir
from concourse.bass_utils import run_bass_kernel_spmd

F32 = mybir.dt.float32
BF16 = mybir.dt.bfloat16
BF = ml_dtypes.bfloat16

B, S, D = 4, 2048, 1024
H = S // 2          # kv-half size (1024)
DT = D // 128       # 8 contraction tiles
ET = D // 128       # 8 output-dim tiles
XCH = H // 512      # 2 column chunks of xh
NCH = S // 512      # 4 query chunks
N_CORES = 8

_compiled = None


def _build():
    nc = bacc.Bacc("TRN2", target_bir_lowering=False, debug=False,
                   num_devices=N_CORES)

    xh = nc.dram_tensor("xh", [D, H], BF16, kind="ExternalInput").ap()
    wqt = nc.dram_tensor("wqt", [D, D], BF16, kind="ExternalInput").ap()
    wkt = nc.dram_tensor("wkt", [D, D], BF16, kind="ExternalInput").ap()
    wvt = nc.dram_tensor("wvt", [D, D], BF16, kind="ExternalInput").ap()
    bq1 = nc.dram_tensor("bq1", [D], F32, kind="ExternalInput").ap()
    bk1 = nc.dram_tensor("bk1", [D], F32, kind="ExternalInput").ap()
    ones = nc.dram_tensor("ones", [128, 1], BF16, kind="ExternalInput").ap()

    accT_d = nc.dram_tensor("accT_d", [D, S], BF16, kind="ExternalOutput").ap()
    l_d = nc.dram_tensor("l_d", [S], F32, kind="ExternalOutput").ap()

    Ident = mybir.ActivationFunctionType.Identity
    Exp = mybir.ActivationFunctionType.Exp

    with tile.TileContext(nc) as tc:
        with (
            tc.tile_pool(name="const", bufs=1) as const,
            tc.tile_pool(name="wpool", bufs=1) as wpool,
            tc.tile_pool(name="xpool", bufs=1) as xpool,
            tc.tile_pool(name="qtp", bufs=1) as qtp,
            tc.tile_pool(name="ktp", bufs=1) as ktp,
            tc.tile_pool(name="vvp", bufs=1) as vvp,
            tc.tile_pool(name="ptp", bufs=1) as ptp,
            tc.tile_pool(name="stg", bufs=4) as stg,
            tc.tile_pool(name="lst", bufs=2) as lst,
        ):
            ones_sb = const.tile([128, 1], BF16, name="ones_sb")
            bq_sb = const.tile([128, ET], F32, name="bq_sb")
            bk_sb = const.tile([128, ET], F32, name="bk_sb")

            wq_t = [wpool.tile([128, D], BF16, name=f"wq{dt}")
                    for dt in range(DT)]
            wk_t = [wpool.tile([128, D], BF16, name=f"wk{dt}")
                    for dt in range(DT)]
            wv_t = [wpool.tile([128, D], BF16, name=f"wv{dt}")
                    for dt in range(DT)]
            xct = [[xpool.tile([128, 512], BF16, name=f"x{c}_{dt}")
                    for dt in range(DT)] for c in range(XCH)]
            qt_c = [qtp.tile([128, ET, 512], BF16, name=f"qt{c}")
                    for c in range(NCH)]
            kt_c = [ktp.tile([128, ET, 512], BF16, name=f"kt{cc}")
                    for cc in range(XCH)]
            v_c = [vvp.tile([128, 4, D], BF16, name=f"v{cc}")
                   for cc in range(XCH)]
            pt_b = [ptp.tile([128, 8, 512], BF16, name=f"pt{blk}")
                    for blk in range(NCH)]

            # DRAM bounce buffers for the pairwise Q AllGather, split per
            # 512-query chunk so the first exchange starts ~15us earlier
            with tc.tile_pool(name="dram", bufs=1, space="DRAM") as dram:
                qown_c = [dram.tile([D, 512], BF16, name=f"qown{c}")
                          for c in range(XCH)]
                qall_c = [dram.tile([2, D, 512], BF16, name=f"qall{c}")
                          for c in range(XCH)]

            # core parity (runtime register) for the predicated gather-in
            parity = nc.sync.partition_id() & 1
            not_parity = 1 - parity

            # ---- input DMAs in deadline order (SP processes serially,
            # ~185GB/s; program order IS the schedule) ----
            def dma_w(wt, src, dt):
                for p in range(2):
                    nc.sync.dma_start(
                        out=wt[dt][:, p * 512:(p + 1) * 512],
                        in_=src[dt * 128:(dt + 1) * 128,
                                p * 512:(p + 1) * 512])

            # first-needed pieces lead the stream; tiny const DMAs
            # (~800ns fixed cost each) follow so they don't delay them
            dma_w(wq_t, wqt, 0)
            nc.sync.dma_start(out=xct[0][0], in_=xh[0:128, 0:512])
            nc.sync.dma_start(out=ones_sb, in_=ones)
            nc.sync.dma_start(
                out=bq_sb,
                in_=bass.AP(tensor=bq1.tensor, offset=0,
                            ap=[[1, 128], [128, ET]]))
            nc.sync.dma_start(
                out=bk_sb,
                in_=bass.AP(tensor=bk1.tensor, offset=0,
                            ap=[[1, 128], [128, ET]]))
            for dt in range(1, DT):
                dma_w(wq_t, wqt, dt)
                nc.sync.dma_start(out=xct[0][dt],
                                  in_=xh[dt * 128:(dt + 1) * 128, 0:512])
            for dt in range(DT):
                nc.sync.dma_start(out=xct[1][dt],
                                  in_=xh[dt * 128:(dt + 1) * 128, 512:1024])
            for dt in range(DT):
                dma_w(wk_t, wkt, dt)

            # ---- PE warmup sized to the initial DMA window: ~12 cold
            # (1.2GHz) junk matmuls end right as the first inputs land,
            # so phase A starts with the HAM clock gate already at 8/8.
            warm = const.tile([128, 512], BF16, name="warm")
            nc.vector.memset(warm, 0.0)
            with tc.tile_pool(name="psW", bufs=1, space="PSUM") as psW:
                wps = psW.tile([128, 512], F32, tag="w", name="wps")
                for i in range(12):
                    nc.tensor.matmul(wps, warm[:, 0:128], warm,
                                     start=True, stop=True)

            # ================= Phases A/B: projections =================
            with tc.tile_pool(name="psAB", bufs=8, space="PSUM") as psA:
                # ---- Phase A: Q projection for the OWN kv-half queries;
                # each chunk is bounced to DRAM for the pair AllGather.
                for c in range(XCH):
                    ps = [psA.tile([128, 512], F32, tag="ps",
                                   name=f"psq{c}_{e}") for e in range(ET)]
                    for dt in range(DT):
                        for e in range(ET):
                            nc.tensor.matmul(
                                ps[e], wq_t[dt][:, e * 128:(e + 1) * 128],
                                xct[c][dt],
                                start=(dt == 0), stop=(dt == DT - 1))
                            # inline drain right after each e's stop-matmul
                            # so the banks free up before the next chunk
                            if dt == DT - 1:
                                nc.scalar.activation(
                                    qt_c[c][:, e, :], ps[e], Ident,
                                    bias=bq_sb[:, e:e + 1], scale=1.0)
                    for e in range(ET):
                        nc.sync.dma_start(
                            out=qown_c[c][e * 128:(e + 1) * 128, :],
                            in_=qt_c[c][:, e, :])
                    # per-chunk pairwise AllGather, launched as soon as
                    # this chunk's bounce is written; overlaps phase B
                    nc.gpsimd.collective_compute(
                        "AllGather", mybir.AluOpType.bypass,
                        replica_groups=[[0, 1], [2, 3], [4, 5], [6, 7]],
                        ins=[qown_c[c].opt()], outs=[qall_c[c].opt()])

                # wv loads slot between the qown chunks' act-waits
                for dt in range(DT):
                    dma_w(wv_t, wvt, dt)

                # peer half -> qt chunks 2,3.  The source half index is a
                # runtime register (1-parity): even cores read qall_c[1],
                # odd cores qall_c[0].  A single DMA per slice (vs a
                # cond-predicated pair) avoids WAW serialization.
                for c2 in range(2):
                    for e in range(ET):
                        src = qall_c[c2][bass.ts(not_parity, 1),
                                         e * 128:(e + 1) * 128, :]
                        nc.sync.dma_start(out=qt_c[2 + c2][:, e, :],
                                          in_=src)

                # ---- Phase B: K ----
                for cc in range(XCH):
                    ps = [psA.tile([128, 512], F32, tag="ps",
                                   name=f"psk{cc}_{e}") for e in range(ET)]
                    for dt in range(DT):
                        for e in range(ET):
                            nc.tensor.matmul(
                                ps[e], wk_t[dt][:, e * 128:(e + 1) * 128],
                                xct[cc][dt],
                                start=(dt == 0), stop=(dt == DT - 1))
                            if dt == DT - 1:
                                nc.scalar.activation(
                                    kt_c[cc][:, e, :], ps[e], Ident,
                                    bias=bk_sb[:, e:e + 1], scale=1.0)

                # ---- Phase B: V (x-slices stationary, wv moving) ----
                for cc in range(XCH):
                    for j2 in range(4):
                        pv = [psA.tile([128, 512], F32, tag="ps",
                                       name=f"psv{cc}_{j2}_{ec}")
                              for ec in range(2)]
                        for dt in range(DT):
                            for ec in range(2):
                                nc.tensor.matmul(
                                    pv[ec],
                                    xct[cc][dt][:, j2 * 128:(j2 + 1) * 128],
                                    wv_t[dt][:, ec * 512:(ec + 1) * 512],
                                    start=(dt == 0), stop=(dt == DT - 1))
                        for ec in range(2):
                            nc.vector.tensor_copy(
                                v_c[cc][:, j2, ec * 512:(ec + 1) * 512],
                                pv[ec])

            # ================= Phase C: attention =================
            with tc.tile_pool(name="psC", bufs=6, space="PSUM") as psC:
                # l(blk) = ones^T @ PT(blk).  Emitted one block late so its
                # exp dependencies are long done (no PE stall), and it
                # shares the "ps" tag so pool-slot rotation forces it to
                # execute inside the ST stream instead of drifting to the
                # kernel tail.
                def emit_l(blk):
                    lp = psC.tile([128, 512], F32, tag="ps",
                                  name=f"lp{blk}")
                    for j in range(8):
                        nc.tensor.matmul(
                            lp[0:1, :], ones_sb, pt_b[blk][:, j, :],
                            start=(j == 0), stop=(j == 7))
                    l_st = lst.tile([1, 512], F32, tag="l",
                                    name=f"lst{blk}")
                    nc.vector.tensor_copy(l_st, lp[0:1, :])
                    nc.sync.dma_start(
                        out=l_d[blk * 512:(blk + 1) * 512], in_=l_st)

                # ---- ST = K @ Q^T, PT = exp(ST/32); queries in LOCAL
                # order: blocks 0,1 = own half (no collective dep),
                # blocks 2,3 = peer half.
                for blk in range(NCH):
                    for j in range(8):
                        cc, jj = divmod(j, 4)
                        sp = psC.tile([128, 512], F32, tag="ps",
                                      name=f"sp{blk}_{j}")
                        for e in range(ET):
                            nc.tensor.matmul(
                                sp, kt_c[cc][:, e, jj * 128:(jj + 1) * 128],
                                qt_c[blk][:, e, :],
                                start=(e == 0), stop=(e == ET - 1))
                        nc.scalar.activation(
                            pt_b[blk][:, j, :], sp, Exp,
                            bias=0.0, scale=float(1.0 / 32.0))
                    if blk >= 1:
                        emit_l(blk - 1)
                emit_l(NCH - 1)

                # ---- accT = V^T-slices @ PT ----
                # blk-outer so each PSUM bank drains right after its
                # 8-chain (spreads the DVE copies and shortens the tail)
                for e in range(ET):
                    for blk in range(NCH):
                        av = psC.tile([128, 512], F32, tag="ps",
                                      name=f"av{e}_{blk}")
                        for j in range(8):
                            cc, jj = divmod(j, 4)
                            nc.tensor.matmul(
                                av,
                                v_c[cc][:, jj, e * 128:(e + 1) * 128],
                                pt_b[blk][:, j, :],
                                start=(j == 0), stop=(j == 7))
                        st_t = stg.tile([128, 512], BF16, tag="stg",
                                        name=f"acc{e}_{blk}")
                        nc.vector.tensor_copy(st_t, av)
                        nc.sync.dma_start(
                            out=accT_d[e * 128:(e + 1) * 128,
                                       blk * 512:(blk + 1) * 512],
                            in_=st_t)

    nc.compile()
    return nc


def _get_compiled():
    global _compiled
    if _compiled is None:
        _compiled = _build()
    return _compiled


def run_sharded(inputs, **run_kwargs):
    """Build per-core in_maps, run SPMD, return BassKernelResults."""
    x = np.ascontiguousarray(inputs["x"], dtype=np.float32)
    Wq = np.asarray(inputs["Wq"], dtype=np.float32)
    Wk = np.asarray(inputs["Wk"], dtype=np.float32)
    Wv = np.asarray(inputs["Wv"], dtype=np.float32)
    bq = np.asarray(inputs["bq"], dtype=np.float32)
    bk = np.asarray(inputs["bk"], dtype=np.float32)

    nc = _get_compiled()

    wqt = np.ascontiguousarray(Wq.T).astype(BF)
    wkt = np.ascontiguousarray(Wk.T).astype(BF)
    wvt = np.ascontiguousarray(Wv.T).astype(BF)
    ones = np.ones((128, 1), dtype=np.float32).astype(BF)

    in_maps = []
    for core in range(N_CORES):
        b, h = divmod(core, 2)
        xhb = x[b].T[:, h * H:(h + 1) * H]            # [D, H] own kv-half
        in_maps.append(dict(xh=np.ascontiguousarray(xhb).astype(BF),
                            wqt=wqt, wkt=wkt, wvt=wvt,
                            bq1=bq, bk1=bk, ones=ones))

    return run_bass_kernel_spmd(nc, in_maps, core_ids=list(range(N_CORES)),
                                **run_kwargs)


def kernel(**inputs):
    bv = np.asarray(inputs["bv"], dtype=np.float64)
    res = run_sharded(inputs)

    out = np.empty((B, S, D), dtype=np.float32)
    for b in range(B):
        r0 = res.results[b * 2]
        r1 = res.results[b * 2 + 1]
        a0 = np.asarray(r0["accT_d"], dtype=np.float64)       # [D, S]
        a1 = np.asarray(r1["accT_d"], dtype=np.float64)
        # each core's output query order is [own half | peer half];
        # odd cores' halves are swapped relative to natural order
        a1 = np.concatenate([a1[:, H:], a1[:, :H]], axis=1)
        l0 = np.asarray(r0["l_d"], dtype=np.float64)
        l1 = np.asarray(r1["l_d"], dtype=np.float64)
        l1 = np.concatenate([l1[H:], l1[:H]])
        num = a0.T + a1.T
        den = (l0 + l1)[:, None]
        out[b] = (num / den + bv[None, :]).astype(np.float32)
    return out
